# revision 1
# baseline (speedup 1.0000x reference)
"""3-layer LSTM (B=256, T=512, I=128, H=64) + final linear, on 8 TRN2 NeuronCores.

Strategy (data-parallel: batch 256 -> 32 per core; weights replicated):
  - Skew-2 wavefront over the 3 layers: at step s, layer l computes
    timestep t = s - 2l (T+4 steps total).  The inter-layer offset of 2
    gives the layer->layer+1 forward-feed write (H2B) a full step of
    slack, taking it off the per-step critical chain: the recurrent
    matmuls wait only on the same-layer state write (H2A).  V is
    double-buffered by step parity to make that legal.
  - ALL-SIGMOID gates: one fused sigmoid over the whole [128,192] PSUM
    gate tile covers all 4 gates x 3 layers (partition layout: [f;i]
    for cols 0:96, [o;g] for 96:192).  f,i,o come out directly; the g
    rows are pre-doubled so g = 2*sigmoid(2*a_g) - 1 = tanh(a_g).
  - Cell update (stt = scalar_tensor_tensor, a DVE-only instruction):
        P = (Sg - 0.5) * Si         # = i*g/2   (stt)
        Q = Sf * C                  # = f*c     (tensor_tensor, 2x mode)
        C = (P * 2) + Q             # = c_new   (stt)
    then TC = tanh(C) on the ACT engine, and the h update is a plain
    multiply (2x-eligible) fused directly into the V writes:
        Vnext[64:128, :]  = So * TC   # = h  state half (on chain)
        Vcur[0:64, 32:96] = So * TC   # forward feed, consumed at s+2
                                      # (same parity, off chain)
    so no separate U/H2 tiles or tensor copies exist on the chain.
  - Everything 2-byte is fp16 (not bf16): gate/tanh outputs, the cell
    state, V, x, and the weights.  2 bytes keeps the DVE 2x modes; the
    10 mantissa bits (vs bf16's 7) cut the recurrence quantization
    error ~2.5x.  The cell state C is fp16 so Q = Sf*C also runs in
    2x mode; P stays f32 (stt output, free).
  - One fp16 bias matmul (K=6 indicator) initializes all 192 psum
    columns per step (start=True).
  - V [128, 96] fp16: partitions 0:64 hold the input half (h_{l-1}),
    64:128 the state half (h_l); recurrent matmuls are K=128 with
    M=128 fp16 stationaries (FWL-eligible), 6 per step.
  - x is transposed/cast on the host to [I, T*Bc] fp16 and streamed in
    16-step chunks (triple buffered, off the critical chain).
"""
import numpy as np
import ml_dtypes

B, T, I, H = 256, 512, 128, 64
NCORES = 8
BC = B // NCORES            # 32 batch per core
NB = 3 * BC                 # 96
XCHUNK = 16

BF16 = np.float16
_cache = {}

# PyTorch gate row order: i(0:64) f(64:128) g(128:192) o(192:256).
_permA = np.r_[64:128, 0:64]       # [f; i]
_permB = np.r_[192:256, 128:192]   # [o; g]
_sA = np.full(128, 1.0, np.float32)              # f,i rows (sigmoid direct)
_sB = np.r_[np.full(64, 1.0, np.float32),        # o rows (sigmoid direct)
            np.full(64, 2.0, np.float32)]        # g rows x2: g = 2*sig(2a)-1


def _prep_weights(inputs):
    f32 = np.float32
    W = {}
    for l in range(3):
        Wih = inputs[f'W_ih{l}'].astype(f32)
        Whh = inputs[f'W_hh{l}'].astype(f32)
        b = (inputs[f'b_ih{l}'] + inputs[f'b_hh{l}']).astype(f32)
        for perm, s, tag in ((_permA, _sA, 'A'), (_permB, _sB, 'B')):
            # lhsT[k, gate] layout, gate row scale s (g rows x2 for the
            # sigmoid-form tanh identity)
            if l == 0:
                W[f'wx{tag}'] = (Wih[perm].T * s[None, :]).astype(BF16)
                m = np.zeros((128, 128), f32)
                m[64:128, :] = Whh[perm].T * s[None, :]
                W[f'w0{tag}'] = m.astype(BF16)
            else:
                m = np.concatenate([Wih[perm].T, Whh[perm].T], axis=0)
                m = m * s[None, :]
                W[f'w{l}{tag}'] = m.astype(BF16)
            W.setdefault(f'bias{tag}', []).append(b[perm] * s)
    # bvals [6, 128]: rows 0..2 = A-region bias per layer, 3..5 = B-region.
    W['bvals'] = np.stack(W.pop('biasA') + W.pop('biasB')).astype(BF16)
    ind = np.zeros((6, 192), f32)
    for l in range(3):
        ind[l, 32 * l:32 * l + 32] = 1.0
        ind[3 + l, 96 + 32 * l:96 + 32 * l + 32] = 1.0
    W['ind'] = ind.astype(BF16)
    W['wout'] = inputs['W_out'].astype(f32).T.astype(BF16)  # [64, 2]
    return W


def _build_program():
    import concourse.bacc as bacc
    import concourse.tile as tile
    from concourse import mybir

    AF = mybir.ActivationFunctionType
    ALU = mybir.AluOpType
    bf16 = mybir.dt.bfloat16
    fp16 = mybir.dt.float16
    f32 = mybir.dt.float32

    nc = bacc.Bacc(None, target_bir_lowering=False, debug=False)
    xT_d = nc.dram_tensor("xT", [128, T * BC], fp16, kind="ExternalInput")
    wnames = ['wxA', 'wxB', 'w0A', 'w0B', 'w1A', 'w1B', 'w2A', 'w2B']
    wall_d = nc.dram_tensor("wall", [128, 8 * 128 + 2], fp16, kind="ExternalInput")
    fall_d = nc.dram_tensor("fall", [6, 128 + 192], fp16, kind="ExternalInput")
    out_d = nc.dram_tensor("out", [2, BC], f32, kind="ExternalOutput")

    with tile.TileContext(nc) as tc:
        with (
            tc.tile_pool(name="singles", bufs=1) as singles,
            tc.tile_pool(name="xpool", bufs=3) as xpool,
            tc.tile_pool(name="scr", bufs=3) as scr,
            tc.tile_pool(name="psum", bufs=2, space="PSUM") as psum,
            tc.tile_pool(name="psum_o", bufs=1, space="PSUM") as psum_o,
        ):
            wall = singles.tile([128, 8 * 128 + 2], fp16, tag="wall")
            nc.sync.dma_start(out=wall, in_=wall_d[:, :])
            fall = singles.tile([6, 128 + 192], fp16, tag="fall")
            nc.sync.dma_start(out=fall, in_=fall_d[:, :])
            ws = {n: wall[:, 128 * k:128 * (k + 1)] for k, n in enumerate(wnames)}
            wout = wall[0:64, 8 * 128:8 * 128 + 2]
            bvals = fall[:, 0:128]
            ind = fall[:, 128:128 + 192]

            V0 = singles.tile([128, NB], fp16, tag="V0")
            V1 = singles.tile([128, NB], fp16, tag="V1")
            Vb = [V0, V1]
            C2 = singles.tile([64, NB], fp16, tag="C2")
            Pt = singles.tile([64, NB], f32, tag="Pt")
            Qt = singles.tile([64, NB], fp16, tag="Qt")
            nc.vector.memset(V0, 0.0)
            nc.vector.memset(V1, 0.0)
            nc.vector.memset(C2, 0.0)

            wA = {0: ws['w0A'], 1: ws['w1A'], 2: ws['w2A']}
            wB = {0: ws['w0B'], 1: ws['w1B'], 2: ws['w2B']}

            xtile = None
            for s in range(T + 4):
                ls = [l for l in (0, 1, 2) if 0 <= s - 2 * l < T]
                c0, c1 = min(ls) * 32, (max(ls) + 1) * 32
                cs = slice(c0, c1)
                V = Vb[s % 2]          # read buffer for this step's matmuls
                Vn = Vb[(s + 1) % 2]   # state written for step s+1

                if s % XCHUNK == 0 and s < T:
                    nch = min(XCHUNK, T - s)
                    xtile = xpool.tile([128, XCHUNK * BC], fp16, tag="xt")
                    nc.sync.dma_start(
                        out=xtile[:, 0:nch * BC], in_=xT_d[:, s * BC:(s + nch) * BC])

                pA = psum.tile([128, 2 * NB], f32, tag="pA")
                # bias init for all 192 cols (start=True clears the window)
                nc.tensor.matmul(pA, bvals, ind,
                                 start=True, stop=False, skip_group_check=True)
                if 0 in ls:
                    k = (s % XCHUNK) * BC
                    xs = xtile[:, k:k + BC]
                    nc.tensor.matmul(pA[:, 0:32], ws['wxA'], xs,
                                     start=False, stop=False, skip_group_check=True)
                    nc.tensor.matmul(pA[:, 96:128], ws['wxB'], xs,
                                     start=False, stop=False, skip_group_check=True)
                for l in ls:
                    cl = slice(32 * l, 32 * l + 32)
                    clB = slice(96 + 32 * l, 96 + 32 * l + 32)
                    nc.tensor.matmul(pA[:, cl], wA[l], V[:, cl],
                                     start=False, stop=True, skip_group_check=True)
                    nc.tensor.matmul(pA[:, clB], wB[l], V[:, cl],
                                     start=False, stop=True, skip_group_check=True)

                TT = scr.tile([128, 2 * NB], fp16, tag="TT")
                TC = scr.tile([64, NB], fp16, tag="TC")

                # one fused sigmoid over all gates of all layers
                # (f,i,o direct; g = 2*sig(2a)-1 via pre-doubled g rows)
                nc.scalar.activation(TT, pA, AF.Sigmoid)
                # P = (Sg - 0.5) * Si = i*g/2   [shifted write 64:128 -> 0:64]
                nc.vector.scalar_tensor_tensor(
                    Pt, TT[64:128, 96:192], 0.5, TT[64:128, 0:96],
                    ALU.subtract, ALU.mult)
                # Q = Sf * C = f*c   (plain tensor_tensor; state C = c)
                nc.vector.tensor_mul(Qt, TT[0:64, 0:96], C2)
                # C = (P * 2) + Q = c'  (sliced: protects inactive state)
                nc.vector.scalar_tensor_tensor(
                    C2[:, cs], Pt[:, cs], 2.0, Qt[:, cs],
                    ALU.mult, ALU.add)
                # TC = tanh(C) = tanh(c)
                nc.scalar.activation(TC, C2, AF.Tanh)
                # V state half: H2_l = (TO + 1) * TC   [shift 0:64 -> 64:128]
                # written into next step's buffer (consumed at s+1)
                nc.vector.tensor_mul(
                    Vn[64:128, cs], TT[0:64, 96 + c0:96 + c1], TC[:, cs])
                # V input half for layers l+1: consumed at s+2 (same parity),
                # so this write has a full step of slack -- off the chain.
                f0, f1 = c0, min(c1, 64)
                if f0 < f1:
                    nc.vector.tensor_mul(
                        V[0:64, 32 + f0:32 + f1],
                        TT[0:64, 96 + f0:96 + f1], TC[:, f0:f1])

            # final linear on layer-2 h(T-1): written at s=T+3 into Vb[(T+4)%2]
            H2f = singles.tile([64, BC], fp16, tag="H2f")
            nc.vector.tensor_copy(H2f, Vb[(T + 4) % 2][64:128, 64:96])
            po = psum_o.tile([2, BC], f32, tag="po")
            nc.tensor.matmul(po, wout, H2f, start=True, stop=True)
            outT = singles.tile([2, BC], f32, tag="outT")
            nc.scalar.copy(outT, po)
            nc.sync.dma_start(out=out_d[:, :], in_=outT)

    nc.compile()
    return nc


def pack_operands(W):
    wall = np.zeros((128, 8 * 128 + 2), BF16)
    for k, n in enumerate(['wxA', 'wxB', 'w0A', 'w0B', 'w1A', 'w1B', 'w2A', 'w2B']):
        wall[:, 128 * k:128 * (k + 1)] = W[n]
    wall[0:64, 1024:1026] = W['wout']
    fall = np.zeros((6, 128 + 192), BF16)
    fall[:, 0:128] = W['bvals']
    fall[:, 128:320] = W['ind']
    return wall, fall


def make_in_maps(inputs):
    W = _prep_weights(inputs)
    wall, fall = pack_operands(W)
    x = inputs['x'].astype(np.float32)
    in_maps = []
    for c in range(NCORES):
        xc = x[c * BC:(c + 1) * BC]                        # [BC, T, I]
        xT = np.ascontiguousarray(xc.transpose(2, 1, 0).reshape(I, T * BC)).astype(BF16)
        in_maps.append({'xT': xT, 'wall': wall, 'fall': fall})
    return in_maps


def kernel(**inputs):
    from concourse.bass_utils import run_bass_kernel_spmd

    inputs = {k: np.asarray(v) for k, v in inputs.items()}
    if 'nc' not in _cache:
        _cache['nc'] = _build_program()
    nc = _cache['nc']

    in_maps = make_in_maps(inputs)
    res = run_bass_kernel_spmd(nc, in_maps, list(range(NCORES)))
    outs = [res.results[c]['out'].T for c in range(NCORES)]   # each [BC, 2]
    full = np.concatenate(outs, axis=0).astype(np.float32)
    full = full + inputs['b_out'].astype(np.float32)[None, :]
    return full



# revision 7
# speedup vs baseline: 1.0695x; 1.0695x over previous
"""3-layer LSTM (B=256, T=512, I=128, H=64) + final linear, on 8 TRN2 NeuronCores.

Strategy (data-parallel: batch 256 -> 32 per core; weights replicated):
  - Skew-2 wavefront over the 3 layers: at step s, layer l computes
    timestep t = s - 2l (T+4 steps total), so one [128,192] gate tile
    covers all 3 layers x 32 batch.
  - Raw-Bass hand-scheduled streams (no Tile framework): every compute
    instruction carries AT MOST ONE attached semaphore wait, so waits
    resolve in the engine wait queues (~35ns) instead of blocking the
    sequencers with standalone EventSemaphore instructions (~200ns+).
    All WAR/WAW hazards are covered transitively by per-engine program
    order plus the single RAW chain:
        mm_s -> sigma_s -> X,Q,C -> tanh_s -> h_s -> mm_{s+1}
  - Semaphores: s_pe +1/step (last recurrent matmul), s_act +1 per ACT
    instruction (sigma=2s+1, tanh=2s+2), s_dve +1 per DVE instruction,
    dma_w / dma_x for weight and x-chunk DMAs (+16 each).
  - ALL-SIGMOID gates: one fused sigmoid over the whole [128,192] PSUM
    gate tile covers all 4 gates x 3 layers (partition layout: [f;i]
    for cols 0:96, [o;g] for 96:192); g rows pre-doubled so
    g = 2*sigmoid(2a)-1 = tanh(a).
  - Cell + hidden updates are all scalar_tensor_tensor in DVE 4x mode
    (all operands fp16 + SBUF + packed -> 0.25 cycles/elem):
        P = (Sg - 0.5) * Si          # = i*g/2
        Q = (Sf bypass) * C          # = f*c
        C = (P * 2) + Q              # = c'
        Vn[64:128,cs]  = So * tanh(C)   # h state half (on chain)
        Vcur[0:64,...] = So * tanh(C)   # forward feed (consumed s+2)
  - One fp16 bias matmul (K=6 indicator) initializes all 192 psum
    columns per step (start=True); bias + x matmuls fire early (only
    gated by psum-bank WAR / DMA), so just the 6 recurrent matmuls sit
    on the per-step critical chain.
  - x is transposed/cast on the host to [I, T*Bc] fp16 and streamed in
    16-step chunks (triple buffered, off the critical chain).
"""
import numpy as np
import ml_dtypes

B, T, I, H = 256, 512, 128, 64
NCORES = 8
BC = B // NCORES            # 32 batch per core
NB = 3 * BC                 # 96
XCHUNK = 16
NXBUF = 3

BF16 = np.float16
_cache = {}

# PyTorch gate row order: i(0:64) f(64:128) g(128:192) o(192:256).
_permA = np.r_[64:128, 0:64]       # [f; i]
_permB = np.r_[192:256, 128:192]   # [o; g]
_sA = np.full(128, 1.0, np.float32)              # f,i rows (sigmoid direct)
_sB = np.r_[np.full(64, 1.0, np.float32),        # o rows (sigmoid direct)
            np.full(64, 2.0, np.float32)]        # g rows x2: g = 2*sig(2a)-1


def _prep_weights(inputs):
    f32 = np.float32
    W = {}
    for l in range(3):
        Wih = inputs[f'W_ih{l}'].astype(f32)
        Whh = inputs[f'W_hh{l}'].astype(f32)
        b = (inputs[f'b_ih{l}'] + inputs[f'b_hh{l}']).astype(f32)
        for perm, s, tag in ((_permA, _sA, 'A'), (_permB, _sB, 'B')):
            # lhsT[k, gate] layout, gate row scale s (g rows x2 for the
            # sigmoid-form tanh identity)
            if l == 0:
                W[f'wx{tag}'] = (Wih[perm].T * s[None, :]).astype(BF16)
                m = np.zeros((128, 128), f32)
                m[64:128, :] = Whh[perm].T * s[None, :]
                W[f'w0{tag}'] = m.astype(BF16)
            else:
                m = np.concatenate([Wih[perm].T, Whh[perm].T], axis=0)
                m = m * s[None, :]
                W[f'w{l}{tag}'] = m.astype(BF16)
        W.setdefault('biasA', []).append(b[_permA] * _sA)
        W.setdefault('biasB', []).append(b[_permB] * _sB)
    # bvals [6, 128]: rows 0..2 = A-region bias per layer, 3..5 = B-region.
    W['bvals'] = np.stack(W.pop('biasA') + W.pop('biasB')).astype(BF16)
    ind = np.zeros((6, 192), np.float32)
    for l in range(3):
        ind[l, 32 * l:32 * l + 32] = 1.0
        ind[3 + l, 96 + 32 * l:96 + 32 * l + 32] = 1.0
    W['ind'] = ind.astype(BF16)
    W['wout'] = inputs['W_out'].astype(f32).T.astype(BF16)  # [64, 2]
    return W


def _step_meta():
    """Per-step layer activity + python-side semaphore value schedule."""
    meta = []
    dve = 3  # 3 memsets precede the loop on DVE
    for s in range(T + 4):
        ls = [l for l in (0, 1, 2) if 0 <= s - 2 * l < T]
        c0, c1 = min(ls) * 32, (max(ls) + 1) * 32
        f0, f1 = c0, min(c1, 64)
        has_ff = f0 < f1
        nops = 4 + (1 if has_ff else 0)   # X, Q, C, h (+ff)
        meta.append(dict(ls=ls, c0=c0, c1=c1, f0=f0, f1=f1, has_ff=has_ff,
                         dve_before=dve,
                         dve_C=dve + 3,       # s_dve value once C_s is done
                         dve_h=dve + 4,       # s_dve value once h_s is done
                         ))
        dve += nops
    return meta, dve


def _build_program():
    import concourse.bacc as bacc
    import concourse.bass as bass
    from concourse import mybir

    AF = mybir.ActivationFunctionType
    ALU = mybir.AluOpType
    fp16 = mybir.dt.float16
    f32 = mybir.dt.float32

    nc = bacc.Bacc(None, target_bir_lowering=False, debug=False)
    xT_d = nc.dram_tensor("xT", [128, T * BC], fp16, kind="ExternalInput")
    wnames = ['wxA', 'wxB', 'w0A', 'w0B', 'w1A', 'w1B', 'w2A', 'w2B']
    wall_d = nc.dram_tensor("wall", [128, 8 * 128 + 2], fp16, kind="ExternalInput")
    fall_d = nc.dram_tensor("fall", [6, 128 + 192], fp16, kind="ExternalInput")
    out_d = nc.dram_tensor("out", [2, BC], f32, kind="ExternalOutput")

    meta, dve_total = _step_meta()
    NCHUNK = T // XCHUNK

    from contextlib import ExitStack
    with ExitStack() as stack:
        e = stack.enter_context
        wall = e(nc.sbuf_tensor("wall_s", [128, 8 * 128 + 2], fp16))
        fall = e(nc.sbuf_tensor("fall_s", [6, 128 + 192], fp16))
        V0 = e(nc.sbuf_tensor("V0", [128, NB], fp16))
        V1 = e(nc.sbuf_tensor("V1", [128, NB], fp16))
        C2 = e(nc.sbuf_tensor("C2", [64, NB], fp16))
        Pt = e(nc.sbuf_tensor("Pt", [64, NB], fp16))
        Qt = e(nc.sbuf_tensor("Qt", [64, NB], fp16))
        TT0 = e(nc.sbuf_tensor("TT0", [128, 2 * NB], fp16))
        TT1 = e(nc.sbuf_tensor("TT1", [128, 2 * NB], fp16))
        TCt = e(nc.sbuf_tensor("TCt", [64, NB], fp16))
        XB = e(nc.sbuf_tensor("XB", [128, NXBUF * XCHUNK * BC], fp16))
        H2f = e(nc.sbuf_tensor("H2f", [64, BC], fp16))
        outT = e(nc.sbuf_tensor("outT", [2, BC], f32))
        PA0 = e(nc.psum_tensor([128, 2 * NB], f32))
        PA1 = e(nc.psum_tensor([128, 2 * NB], f32))
        PO = e(nc.psum_tensor([2, BC], f32))
        dma_w = e(nc.semaphore("dma_w"))
        dma_x = e(nc.semaphore("dma_x"))
        s_pe = e(nc.semaphore("s_pe"))
        s_act = e(nc.semaphore("s_act"))
        s_dve = e(nc.semaphore("s_dve"))
        block = e(nc.Block())
        Vb = [V0, V1]
        TTb = [TT0, TT1]
        PAb = [PA0, PA1]
        ws = {n: wall[:, 128 * k:128 * (k + 1)] for k, n in enumerate(wnames)}
        wout = wall[0:64, 8 * 128:8 * 128 + 2]
        bvals = fall[:, 0:128]
        ind = fall[:, 128:128 + 192]
        wA = {0: ws['w0A'], 1: ws['w1A'], 2: ws['w2A']}
        wB = {0: ws['w0B'], 1: ws['w1B'], 2: ws['w2B']}

        @block.sync
        def _(sync):
            sync.dma_start(out=wall[:, :], in_=wall_d[:, :]).then_inc(dma_w, 16)
            sync.dma_start(out=fall[:, :], in_=fall_d[:, :]).then_inc(dma_w, 16)
            for c in range(NCHUNK):
                buf = c % NXBUF
                ins = sync.dma_start(
                    out=XB[:, buf * XCHUNK * BC:(buf + 1) * XCHUNK * BC],
                    in_=xT_d[:, c * XCHUNK * BC:(c + 1) * XCHUNK * BC])
                if c >= NXBUF:
                    # buffer reused: wait until chunk c-NXBUF's x-matmuls
                    # (step <= 16*(c-NXBUF)+15) are done on PE.
                    ins.wait_op(s_pe, 16 * (c - NXBUF) + 16, "sem-ge")
                ins.then_inc(dma_x, 16)
            # final output store (after epilogue copy: last ACT instruction)
            sync.dma_start(out=out_d[:, :], in_=outT[:, :]).wait_op(
                s_act, 2 * (T + 4) + 1, "sem-ge").then_inc(dma_w, 16)

        @block.tensor
        def _(tensor):
            # standalone wait so the very first Ldweights (which precedes the
            # matmul instruction that would carry the wait) can't read the
            # weight tiles before their DMA lands.
            tensor.wait_ge(dma_w, 32)
            for s in range(T + 4):
                m = meta[s]
                ls, c0, c1 = m['ls'], m['c0'], m['c1']
                pA = PAb[s % 2]
                # bias matmul: opens the psum bank (start=True).  WAR on the
                # bank: sigma_{s-2} must have finished reading it.
                i = nc.tensor.matmul(pA[:, :], bvals, ind,
                                     start=True, stop=False, skip_group_check=True)
                if s >= 2:
                    i.wait_op(s_act, 2 * (s - 2) + 1, "sem-ge")
                # x matmuls (layer 0), gated only by the chunk DMA
                if 0 in ls:
                    k = (s % XCHUNK) * BC
                    buf = (s // XCHUNK) % NXBUF
                    xs = XB[:, buf * XCHUNK * BC + k:buf * XCHUNK * BC + k + BC]
                    i = nc.tensor.matmul(pA[:, 0:32], ws['wxA'], xs,
                                         start=False, stop=False, skip_group_check=True)
                    if s % XCHUNK == 0:
                        i.wait_op(dma_x, 16 * (s // XCHUNK) + 16, "sem-ge")
                    nc.tensor.matmul(pA[:, 96:128], ws['wxB'], xs,
                                     start=False, stop=False, skip_group_check=True)
                # recurrent matmuls: the chain.  First one waits for h_{s-1}.
                first = True
                last = None
                for l in ls:
                    cl = slice(32 * l, 32 * l + 32)
                    clB = slice(96 + 32 * l, 96 + 32 * l + 32)
                    V = Vb[s % 2]
                    i = nc.tensor.matmul(pA[:, cl], wA[l], V[:, cl],
                                         start=False, stop=True, skip_group_check=True)
                    if first:
                        i.wait_op(s_dve, meta[s - 1]['dve_h'] if s else 3, "sem-ge")
                        first = False
                    last = nc.tensor.matmul(pA[:, clB], wB[l], V[:, cl],
                                            start=False, stop=True, skip_group_check=True)
                last.then_inc(s_pe, 1)
            # epilogue: final linear on layer-2 h(T-1), written at s=T+3
            i = nc.tensor.matmul(PO[:, :], wout, H2f[:, :], start=True, stop=True)
            i.wait_op(s_dve, dve_total + 1, "sem-ge")
            i.then_inc(s_pe, 1)

        @block.scalar
        def _(scalar):
            for s in range(T + 4):
                m = meta[s]
                # one fused sigmoid over all gates of all layers
                nc.scalar.activation(TTb[s % 2][:, :], PAb[s % 2][:, :], AF.Sigmoid) \
                    .wait_op(s_pe, s + 1, "sem-ge").then_inc(s_act, 1)
                # tanh of the fresh cell state
                nc.scalar.activation(TCt[:, :], C2[:, :], AF.Tanh) \
                    .wait_op(s_dve, m['dve_C'], "sem-ge").then_inc(s_act, 1)
            # epilogue: copy final linear out of psum
            nc.scalar.copy(outT[:, :], PO[:, :]) \
                .wait_op(s_pe, T + 5, "sem-ge").then_inc(s_act, 1)

        @block.vector
        def _(vector):
            nc.vector.memset(V0[:, :], 0.0).then_inc(s_dve, 1)
            nc.vector.memset(V1[:, :], 0.0).then_inc(s_dve, 1)
            nc.vector.memset(C2[:, :], 0.0).then_inc(s_dve, 1)
            for s in range(T + 4):
                m = meta[s]
                c0, c1 = m['c0'], m['c1']
                cs = slice(c0, c1)
                TT = TTb[s % 2]
                Vn = Vb[(s + 1) % 2]
                Vc = Vb[s % 2]
                # P = (Sg - 0.5) * Si = i*g/2   [shifted write 64:128 -> 0:64]
                nc.vector.scalar_tensor_tensor(
                    Pt[:, :], TT[64:128, 96:192], 0.5, TT[64:128, 0:96],
                    ALU.subtract, ALU.mult) \
                    .wait_op(s_act, 2 * s + 1, "sem-ge").then_inc(s_dve, 1)
                # Q = Sf * C = f*c
                nc.vector.scalar_tensor_tensor(
                    Qt[:, :], TT[0:64, 0:96], 0.0, C2[:, :],
                    ALU.bypass, ALU.mult).then_inc(s_dve, 1)
                # C = (P * 2) + Q = c'  (sliced: protects inactive state)
                nc.vector.scalar_tensor_tensor(
                    C2[:, cs], Pt[:, cs], 2.0, Qt[:, cs],
                    ALU.mult, ALU.add).then_inc(s_dve, 1)
                # V state half: h = So * tanh(C)  [shift 0:64 -> 64:128]
                nc.vector.scalar_tensor_tensor(
                    Vn[64:128, cs], TT[0:64, 96 + c0:96 + c1], 0.0, TCt[:, cs],
                    ALU.bypass, ALU.mult) \
                    .wait_op(s_act, 2 * s + 2, "sem-ge").then_inc(s_dve, 1)
                # forward feed for layers l+1 (consumed at s+2, same parity)
                if m['has_ff']:
                    f0, f1 = m['f0'], m['f1']
                    nc.vector.scalar_tensor_tensor(
                        Vc[0:64, 32 + f0:32 + f1],
                        TT[0:64, 96 + f0:96 + f1], 0.0, TCt[:, f0:f1],
                        ALU.bypass, ALU.mult).then_inc(s_dve, 1)
            # epilogue: stage layer-2 h(T-1) for the final matmul
            nc.vector.tensor_copy(H2f[:, :], Vb[(T + 4) % 2][64:128, 64:96]) \
                .then_inc(s_dve, 1)

    nc.compile()
    return nc


def pack_operands(W):
    wall = np.zeros((128, 8 * 128 + 2), BF16)
    for k, n in enumerate(['wxA', 'wxB', 'w0A', 'w0B', 'w1A', 'w1B', 'w2A', 'w2B']):
        wall[:, 128 * k:128 * (k + 1)] = W[n]
    wall[0:64, 1024:1026] = W['wout']
    fall = np.zeros((6, 128 + 192), BF16)
    fall[:, 0:128] = W['bvals']
    fall[:, 128:320] = W['ind']
    return wall, fall


def make_in_maps(inputs):
    W = _prep_weights(inputs)
    wall, fall = pack_operands(W)
    x = inputs['x'].astype(np.float32)
    in_maps = []
    for c in range(NCORES):
        xc = x[c * BC:(c + 1) * BC]                        # [BC, T, I]
        xT = np.ascontiguousarray(xc.transpose(2, 1, 0).reshape(I, T * BC)).astype(BF16)
        in_maps.append({'xT': xT, 'wall': wall, 'fall': fall})
    return in_maps


def kernel(**inputs):
    from concourse.bass_utils import run_bass_kernel_spmd

    inputs = {k: np.asarray(v) for k, v in inputs.items()}
    if 'nc' not in _cache:
        _cache['nc'] = _build_program()
    nc = _cache['nc']

    in_maps = make_in_maps(inputs)
    res = run_bass_kernel_spmd(nc, in_maps, list(range(NCORES)))
    outs = [res.results[c]['out'].T for c in range(NCORES)]   # each [BC, 2]
    full = np.concatenate(outs, axis=0).astype(np.float32)
    full = full + inputs['b_out'].astype(np.float32)[None, :]
    return full


# revision 13
# speedup vs baseline: 1.2578x; 1.1760x over previous
"""3-layer LSTM (B=256, T=512, I=128, H=64) + final linear, on 8 TRN2 NeuronCores.

G=2 pipelined variant: each core's 32-batch is split into two independent
16-batch chains (A, B) offset by OFFSET ns.  Each chain runs the skew-2
3-layer wavefront with its own gate tile [128, 96], cell state [64, 48],
PSUM bank pair, and semaphores; the two chains share the engines, weights
and x-chunk DMAs.  While chain A sits in a cross-engine latency gap
(write-commit tails + semaphore props dominate the serial recurrence),
chain B's work executes on the idle engines.

Per-engine instruction emission follows the nominal steady-state schedule
(period PERIOD, chain B at +OFFSET) because engine queues are FIFO with
head-of-line blocking: emission order must equal execution order.

Sync design (per chain): every instruction carries at most ONE attached
sem wait; WAR/WAW covered transitively by engine order; same-engine RAW
(X,Q -> C) gets an explicit self-wait because DVE writes commit only
after the pipeline drain.
"""
import numpy as np
import ml_dtypes

B, T, I, H = 256, 512, 128, 64
NCORES = 8
BC = B // NCORES            # 32 batch per core
G = 2
BG = BC // G                # 16 batch per chain
NB = 3 * BG                 # 48 cols per chain
XCHUNK = 16
NXBUF = 3

PERIOD = 1856.0
OFFSET = 480.0

BF16 = np.float16
_cache = {}

_permA = np.r_[64:128, 0:64]       # [f; i]
_permB = np.r_[192:256, 128:192]   # [o; g]
_sA = np.full(128, 1.0, np.float32)
_sB = np.r_[np.full(64, 1.0, np.float32),
            np.full(64, 2.0, np.float32)]


def _prep_weights(inputs):
    f32 = np.float32
    W = {}
    for l in range(3):
        Wih = inputs[f'W_ih{l}'].astype(f32)
        Whh = inputs[f'W_hh{l}'].astype(f32)
        b = (inputs[f'b_ih{l}'] + inputs[f'b_hh{l}']).astype(f32)
        for perm, s, tag in ((_permA, _sA, 'A'), (_permB, _sB, 'B')):
            if l == 0:
                W[f'wx{tag}'] = (Wih[perm].T * s[None, :]).astype(BF16)
                m = np.zeros((128, 128), f32)
                m[64:128, :] = Whh[perm].T * s[None, :]
                W[f'w0{tag}'] = m.astype(BF16)
            else:
                m = np.concatenate([Wih[perm].T, Whh[perm].T], axis=0)
                m = m * s[None, :]
                W[f'w{l}{tag}'] = m.astype(BF16)
        W.setdefault('biasA', []).append(b[_permA] * _sA)
        W.setdefault('biasB', []).append(b[_permB] * _sB)
    W['bvals'] = np.stack(W.pop('biasA') + W.pop('biasB')).astype(BF16)
    # per-chain indicator [6, 2*NB]: layer l -> cols 16l:16l+16 (A region)
    # and NB+16l:... (B region)
    ind = np.zeros((6, 2 * NB), np.float32)
    for l in range(3):
        ind[l, BG * l:BG * l + BG] = 1.0
        ind[3 + l, NB + BG * l:NB + BG * l + BG] = 1.0
    W['ind'] = ind.astype(BF16)
    W['wout'] = inputs['W_out'].astype(f32).T.astype(BF16)  # [64, 2]
    return W


def _step_meta():
    meta = []
    dve = 3  # memsets (V0, V1, D) per chain happen on chain A=0/B=1 counters
    for s in range(T + 4):
        ls = [l for l in (0, 1, 2) if 0 <= s - 2 * l < T]
        c0, c1 = min(ls) * BG, (max(ls) + 1) * BG
        f0, f1 = c0, min(c1, 2 * BG)
        has_ff = f0 < f1
        nops = 4 + (1 if has_ff else 0)
        meta.append(dict(ls=ls, c0=c0, c1=c1, f0=f0, f1=f1, has_ff=has_ff,
                         dve_before=dve, dve_Q=dve + 2, dve_C=dve + 3,
                         dve_h=dve + 4))
        dve += nops
    return meta, dve


def _build_program():
    import concourse.bacc as bacc
    from concourse import mybir

    AF = mybir.ActivationFunctionType
    ALU = mybir.AluOpType
    fp16 = mybir.dt.float16
    f32 = mybir.dt.float32

    nc = bacc.Bacc(None, target_bir_lowering=False, debug=False)
    xT_d = nc.dram_tensor("xT", [128, T * BC], fp16, kind="ExternalInput")
    wnames = ['wxA', 'wxB', 'w0A', 'w0B', 'w1A', 'w1B', 'w2A', 'w2B']
    wall_d = nc.dram_tensor("wall", [128, 8 * 128 + 2], fp16, kind="ExternalInput")
    fall_d = nc.dram_tensor("fall", [6, 128 + 2 * NB], fp16, kind="ExternalInput")
    out_d = nc.dram_tensor("out", [2, BC], f32, kind="ExternalOutput")

    meta, dve_total = _step_meta()
    NCHUNK = T // XCHUNK

    from contextlib import ExitStack
    with ExitStack() as stack:
        e = stack.enter_context
        wall = e(nc.sbuf_tensor("wall_s", [128, 8 * 128 + 2], fp16))
        fall = e(nc.sbuf_tensor("fall_s", [6, 128 + 2 * NB], fp16))
        XB = e(nc.sbuf_tensor("XB", [128, NXBUF * XCHUNK * BC], fp16))
        H2f = e(nc.sbuf_tensor("H2f", [64, BC], fp16))
        outT = e(nc.sbuf_tensor("outT", [2, BC], f32))
        PO = e(nc.psum_tensor([2, BC], f32))
        dma_w = e(nc.semaphore("dma_w"))
        dma_x = e(nc.semaphore("dma_x"))
        s_xc = e(nc.semaphore("s_xc"))
        s_ep = e(nc.semaphore("s_ep"))
        ch = []
        for g in range(G):
            V0 = e(nc.sbuf_tensor(f"V0_{g}", [128, NB], fp16))
            V1 = e(nc.sbuf_tensor(f"V1_{g}", [128, NB], fp16))
            Dt = e(nc.sbuf_tensor(f"D_{g}", [64, NB], fp16))
            Pt = e(nc.sbuf_tensor(f"Pt_{g}", [64, NB], fp16))
            Qt = e(nc.sbuf_tensor(f"Qt_{g}", [64, NB], fp16))
            TT0 = e(nc.sbuf_tensor(f"TT0_{g}", [128, 2 * NB], fp16))
            TT1 = e(nc.sbuf_tensor(f"TT1_{g}", [128, 2 * NB], fp16))
            TC = e(nc.sbuf_tensor(f"TC_{g}", [64, NB], fp16))
            PA0 = e(nc.psum_tensor([128, 2 * NB], f32))
            PA1 = e(nc.psum_tensor([128, 2 * NB], f32))
            s_pe = e(nc.semaphore(f"s_pe{g}"))
            s_act = e(nc.semaphore(f"s_act{g}"))
            s_dve = e(nc.semaphore(f"s_dve{g}"))
            d = dict(V0=V0, V1=V1, D=Dt, Pt=Pt, Qt=Qt, TT0=TT0, TT1=TT1,
                     TC=TC, PA0=PA0, PA1=PA1, s_pe=s_pe, s_act=s_act,
                     s_dve=s_dve)
            d['Vb'] = [V0, V1]
            d['TTb'] = [TT0, TT1]
            d['PAb'] = [PA0, PA1]
            ch.append(d)
        block = e(nc.Block())

        ws = {n: wall[:, 128 * k:128 * (k + 1)] for k, n in enumerate(wnames)}
        wout = wall[0:64, 8 * 128:8 * 128 + 2]
        bvals = fall[:, 0:128]
        ind = fall[:, 128:128 + 2 * NB]
        wA = {0: ws['w0A'], 1: ws['w1A'], 2: ws['w2A']}
        wB = {0: ws['w0B'], 1: ws['w1B'], 2: ws['w2B']}

        # ---- nominal schedule: (time, emit_closure) per engine ------------
        sched = {'pe': [], 'act': [], 'dve': []}

        def at(engine, t, fn):
            sched[engine].append((t, len(sched[engine]), fn))

        def t_of(g, s, off):
            return PERIOD * s + OFFSET * g + off

        for g in range(G):
            c = ch[g]
            for s in range(T + 4):
                m = meta[s]
                at('pe', t_of(g, s - 2, 1684), _mk_bias(nc, c, meta, s, bvals, ind, dma_w))
                if 0 in m['ls']:
                    at('pe', t_of(g, s - 2, 1724),
                       _mk_xmm(nc, c, s, g, ws, XB, dma_x, s_xc))
                at('pe', t_of(g, s - 1, 1604), _mk_rec(nc, c, meta, s, wA, wB))
                at('act', t_of(g, s, 0), _mk_sigma(nc, c, s, AF))
                at('act', t_of(g, s, 957), _mk_tanh(nc, c, meta, s, AF))
                at('dve', t_of(g, s, 483), _mk_X(nc, c, s, ALU))
                at('dve', t_of(g, s, 593), _mk_Q(nc, c, s))
                at('dve', t_of(g, s, 774), _mk_C(nc, c, meta, s))
                at('dve', t_of(g, s, 1400), _mk_h(nc, c, meta, s))
                if m['has_ff']:
                    at('dve', t_of(g, s, 1485), _mk_ff(nc, c, meta, s))
            # epilogue copy for this chain
            at('dve', t_of(g, T + 4, 100), _mk_epcopy(nc, c, g, H2f, dve_total, s_ep))
        # final matmul + copy out
        at('pe', t_of(1, T + 5, 0), _mk_epmm(nc, PO, wout, H2f, s_ep, ch))
        at('act', t_of(1, T + 5, 500), _mk_epout(nc, outT, PO, ch))

        for k in sched:
            sched[k].sort(key=lambda x: (x[0], x[1]))

        @block.sync
        def _(sync):
            sync.dma_start(out=wall[:, :], in_=wall_d[:, :]).then_inc(dma_w, 16)
            sync.dma_start(out=fall[:, :], in_=fall_d[:, :]).then_inc(dma_w, 16)
            for cc in range(NCHUNK):
                buf = cc % NXBUF
                ins = sync.dma_start(
                    out=XB[:, buf * XCHUNK * BC:(buf + 1) * XCHUNK * BC],
                    in_=xT_d[:, cc * XCHUNK * BC:(cc + 1) * XCHUNK * BC])
                if cc >= NXBUF:
                    # both chains must have consumed chunk cc-NXBUF
                    ins.wait_op(s_xc, G * (cc - NXBUF + 1), "sem-ge")
                ins.then_inc(dma_x, 16)
            sync.dma_start(out=out_d[:, :], in_=outT[:, :]).wait_op(
                ch[0]['s_act'], 2 * (T + 4) + 1, "sem-ge").then_inc(dma_w, 16)

        @block.tensor
        def _(tensor):
            tensor.wait_ge(dma_w, 32)
            for t, k, fn in sched['pe']:
                fn()

        @block.scalar
        def _(scalar):
            for t, k, fn in sched['act']:
                fn()

        @block.vector
        def _(vector):
            for g in range(G):
                c = ch[g]
                nc.vector.memset(c['V0'][:, :], 0.0).then_inc(c['s_dve'], 1)
                nc.vector.memset(c['V1'][:, :], 0.0).then_inc(c['s_dve'], 1)
                nc.vector.memset(c['D'][:, :], 0.0).then_inc(c['s_dve'], 1)
            for t, k, fn in sched['dve']:
                fn()

    nc.compile()
    return nc


# ---- emit closures (default-arg capture) ----------------------------------

def _mk_bias(nc, c, meta, s, bvals, ind, dma_w):
    def f():
        i = nc.tensor.matmul(c['PAb'][s % 2][:, :], bvals, ind,
                             start=True, stop=False, skip_group_check=True)
        if s >= 2:
            i.wait_op(c['s_act'], 2 * (s - 2) + 1, "sem-ge")
    return f


def _mk_xmm(nc, c, s, g, ws, XB, dma_x, s_xc):
    def f():
        cc = s // XCHUNK
        buf = cc % NXBUF
        k = (s % XCHUNK) * BC + g * BG
        xs = XB[:, buf * XCHUNK * BC + k:buf * XCHUNK * BC + k + BG]
        pA = c['PAb'][s % 2]
        i = nc.tensor.matmul(pA[:, 0:BG], ws['wxA'], xs,
                             start=False, stop=False, skip_group_check=True)
        if s % XCHUNK == 0:
            i.wait_op(dma_x, 16 * (cc + 1), "sem-ge")
        i2 = nc.tensor.matmul(pA[:, NB:NB + BG], ws['wxB'], xs,
                              start=False, stop=False, skip_group_check=True)
        if s % XCHUNK == XCHUNK - 1 or s == T - 1:
            i2.then_inc(s_xc, 1)
    return f


def _mk_rec(nc, c, meta, s, wA, wB):
    def f():
        m = meta[s]
        pA = c['PAb'][s % 2]
        V = c['Vb'][s % 2]
        first = True
        last = None
        for l in m['ls']:
            cl = slice(BG * l, BG * l + BG)
            clB = slice(NB + BG * l, NB + BG * l + BG)
            i = nc.tensor.matmul(pA[:, cl], wA[l], V[:, cl],
                                 start=False, stop=True, skip_group_check=True)
            if first:
                i.wait_op(c['s_dve'], meta[s - 1]['dve_h'] if s else 3, "sem-ge")
                first = False
            last = nc.tensor.matmul(pA[:, clB], wB[l], V[:, cl],
                                    start=False, stop=True, skip_group_check=True)
        last.then_inc(c['s_pe'], 1)
    return f


def _mk_sigma(nc, c, s, AF):
    def f():
        nc.scalar.activation(c['TTb'][s % 2][:, :], c['PAb'][s % 2][:, :],
                             AF.Sigmoid) \
            .wait_op(c['s_pe'], s + 1, "sem-ge").then_inc(c['s_act'], 1)
    return f


def _mk_tanh(nc, c, meta, s, AF):
    def f():
        nc.scalar.activation(c['TC'][:, :], c['D'][:, :], AF.Tanh, scale=2.0) \
            .wait_op(c['s_dve'], meta[s]['dve_C'], "sem-ge").then_inc(c['s_act'], 1)
    return f


def _mk_X(nc, c, s, ALU):
    def f():
        TT = c['TTb'][s % 2]
        nc.vector.scalar_tensor_tensor(
            c['Pt'][:, :], TT[64:128, NB:2 * NB], 0.5, TT[64:128, 0:NB],
            ALU.subtract, ALU.mult) \
            .wait_op(c['s_act'], 2 * s + 1, "sem-ge").then_inc(c['s_dve'], 1)
    return f


def _mk_Q(nc, c, s):
    def f():
        TT = c['TTb'][s % 2]
        nc.vector.tensor_mul(
            c['Qt'][:, :], TT[0:64, 0:NB], c['D'][:, :]).then_inc(c['s_dve'], 1)
    return f


def _mk_C(nc, c, meta, s):
    def f():
        m = meta[s]
        cs = slice(m['c0'], m['c1'])
        nc.vector.tensor_add(
            c['D'][:, cs], c['Pt'][:, cs], c['Qt'][:, cs]) \
            .wait_op(c['s_dve'], m['dve_Q'], "sem-ge").then_inc(c['s_dve'], 1)
    return f


def _mk_h(nc, c, meta, s):
    def f():
        m = meta[s]
        cs = slice(m['c0'], m['c1'])
        TT = c['TTb'][s % 2]
        nc.vector.tensor_mul(
            c['Vb'][(s + 1) % 2][64:128, cs],
            TT[0:64, NB + m['c0']:NB + m['c1']], c['TC'][:, cs]) \
            .wait_op(c['s_act'], 2 * s + 2, "sem-ge").then_inc(c['s_dve'], 1)
    return f


def _mk_ff(nc, c, meta, s):
    def f():
        m = meta[s]
        f0, f1 = m['f0'], m['f1']
        TT = c['TTb'][s % 2]
        nc.vector.tensor_mul(
            c['Vb'][s % 2][0:64, BG + f0:BG + f1],
            TT[0:64, NB + f0:NB + f1], c['TC'][:, f0:f1]).then_inc(c['s_dve'], 1)
    return f


def _mk_epcopy(nc, c, g, H2f, dve_total, s_ep):
    def f():
        nc.vector.tensor_copy(
            H2f[:, g * BG:(g + 1) * BG],
            c['Vb'][(T + 4) % 2][64:128, 2 * BG:3 * BG]) \
            .wait_op(c['s_dve'], dve_total, "sem-ge").then_inc(s_ep, 1)
    return f


def _mk_epmm(nc, PO, wout, H2f, s_ep, ch):
    def f():
        i = nc.tensor.matmul(PO[:, :], wout, H2f[:, :], start=True, stop=True)
        i.wait_op(s_ep, G, "sem-ge")
        i.then_inc(ch[0]['s_pe'], 1)
    return f


def _mk_epout(nc, outT, PO, ch):
    def f():
        nc.scalar.copy(outT[:, :], PO[:, :]) \
            .wait_op(ch[0]['s_pe'], T + 5, "sem-ge").then_inc(ch[0]['s_act'], 1)
    return f


def pack_operands(W):
    wall = np.zeros((128, 8 * 128 + 2), BF16)
    for k, n in enumerate(['wxA', 'wxB', 'w0A', 'w0B', 'w1A', 'w1B', 'w2A', 'w2B']):
        wall[:, 128 * k:128 * (k + 1)] = W[n]
    wall[0:64, 1024:1026] = W['wout']
    fall = np.zeros((6, 128 + 2 * NB), BF16)
    fall[:, 0:128] = W['bvals']
    fall[:, 128:128 + 2 * NB] = W['ind']
    return wall, fall


def make_in_maps(inputs):
    W = _prep_weights(inputs)
    wall, fall = pack_operands(W)
    x = inputs['x'].astype(np.float32)
    in_maps = []
    for c in range(NCORES):
        xc = x[c * BC:(c + 1) * BC]
        xT = np.ascontiguousarray(xc.transpose(2, 1, 0).reshape(I, T * BC)).astype(BF16)
        in_maps.append({'xT': xT, 'wall': wall, 'fall': fall})
    return in_maps


def kernel(**inputs):
    from concourse.bass_utils import run_bass_kernel_spmd

    inputs = {k: np.asarray(v) for k, v in inputs.items()}
    if 'nc' not in _cache:
        _cache['nc'] = _build_program()
    nc = _cache['nc']

    in_maps = make_in_maps(inputs)
    res = run_bass_kernel_spmd(nc, in_maps, list(range(NCORES)))
    outs = [res.results[c]['out'].T for c in range(NCORES)]
    full = np.concatenate(outs, axis=0).astype(np.float32)
    full = full + inputs['b_out'].astype(np.float32)[None, :]
    return full
